# revision 1
# baseline (speedup 1.0000x reference)
"""Mixture-of-Softmaxes with shared embedding — 8-core Trainium2 Bass kernel.

Strategy (tensor-parallel on the vocab output head, per the sharding hint):
  - Vocab dim V is sharded across the 8 cores (Vp = 6283 rows each, zero-padded
    from 50257 to 50264; the 7 pad rows contribute exactly exp(0)=1 to each
    softmax denominator and are corrected by a constant subtraction).
  - The expert transforms (10 experts x 2560x2560) are sharded as 200
    (expert, d-block) jobs, 25 per core, followed by an AllGather of the
    bf16 expert_hidden^T (13 MB).
  - Softmax over the full vocab needs a cross-core reduction: since the
    logits are O(1) in magnitude, exp() cannot overflow in fp32, so the
    max-shift is unnecessary and the reduction collapses to a single
    AllReduce-ADD of the (256,10) sum-of-exp stats.
  - RMSNorm is folded in linearly: norm_scale is folded into the expert and
    gate weights on the host; the per-token 1/rms factor is computed on-chip
    in fp32 and folded into the bf16 h^T operand.

kernel(**inputs) takes the full unsharded inputs and returns the full
(1, 256, 50257) float32 logits.
"""
import sys

for _p in ("/opt/trn_rl_repo",):
    if _p not in sys.path:
        sys.path.append(_p)

import numpy as np
import ml_dtypes

import concourse.bass as bass
import concourse.bacc as bacc
import concourse.mybir as mybir
import concourse.tile as tile
from concourse.bass_utils import run_bass_kernel_spmd

BF16 = ml_dtypes.bfloat16

NCORES = 8
S = 256          # tokens (B*S)
H = 2560         # hidden
E = 10           # experts
V = 50257        # vocab
KB = H // 128    # 20 k-blocks
NJOBS = E * KB   # 200 (expert, d-block) jobs
JPC = NJOBS // NCORES  # 25 jobs per core
VP = 6283        # per-core vocab slice (8*6283 = 50264)
NPAD = NCORES * VP - V  # 7 zero-pad vocab rows (on the last core)
CHUNK = 512
CHUNKS = [(i * CHUNK, CHUNK) for i in range(VP // CHUNK)]
if VP % CHUNK:
    CHUNKS.append((VP - VP % CHUNK, VP % CHUNK))
EPS_NORM = 1e-05
EPS_LOG = 1e-10

_nc_cache = None


def build_kernel():
    global _nc_cache
    if _nc_cache is not None:
        return _nc_cache
    f32 = mybir.dt.float32
    bf = mybir.dt.bfloat16
    nc = bacc.Bacc("TRN2", target_bir_lowering=False, debug=False, num_devices=NCORES)

    h32 = nc.declare_dram_parameter("h32", [2, 128, H], f32, isOutput=False)
    hT = nc.declare_dram_parameter("hT", [KB, 128, S], bf, isOutput=False)
    gw = nc.declare_dram_parameter("gw", [KB, 128, E], bf, isOutput=False)
    wj = nc.declare_dram_parameter("wjobs", [JPC, KB, 128, 128], bf, isOutput=False)
    embT = nc.declare_dram_parameter("embT", [KB, 128, VP], bf, isOutput=False)
    out = nc.declare_dram_parameter("out", [S, VP], f32, isOutput=True)

    rb = nc.dram_tensor("rbounce", [S], f32)
    ehl = nc.dram_tensor("eh_local", [JPC, 128, S], bf)
    eha = nc.dram_tensor("eh_all", [NJOBS, 128, S], bf, addr_space="Shared")
    zl = [nc.dram_tensor(f"zl{sh}", [128, E], f32) for sh in range(2)]
    za = [
        nc.dram_tensor(f"za{sh}", [128, E], f32, addr_space="Shared")
        for sh in range(2)
    ]
    xsp = nc.dram_tensor("xspill", [2 * E, 128, VP], bf)

    rg = [list(range(NCORES))]

    with tile.TileContext(nc) as tc:
        with (
            tc.tile_pool(name="et", bufs=2) as etp,          # 20KB slots
            tc.tile_pool(name="stream", bufs=3) as strm,     # 10KB slots
            tc.tile_pool(name="xs", bufs=2) as xsp_pool,     # 10KB slots
            tc.tile_pool(name="ehsh", bufs=1) as ehp,        # 51.2KB
            tc.tile_pool(name="acc", bufs=2) as accp,
            tc.tile_pool(name="ot", bufs=2) as otp,
            tc.tile_pool(name="ehl", bufs=4) as ehlp,
            tc.tile_pool(name="persist", bufs=1) as per,
            tc.tile_pool(name="psmall", bufs=4, space="PSUM") as psS,
            tc.tile_pool(name="psbig", bufs=4, space="PSUM") as psC,
        ):
            epsn = per.tile([128, 1], f32, tag="epsn")
            nc.vector.memset(epsn, EPS_NORM)
            epsl = per.tile([128, 1], f32, tag="epsl")
            nc.vector.memset(epsl, EPS_LOG)

            # ---- P0: per-token RMS factors r_s = 1/sqrt(mean(h^2)+eps) ----
            r = []
            for sh in range(2):
                ht = etp.tile([128, H], f32, tag="et")
                nc.sync.dma_start(out=ht, in_=h32[sh])
                sqt = etp.tile([128, H], f32, tag="et")
                nc.vector.tensor_mul(sqt, ht, ht)
                sq = per.tile([128, 1], f32, tag=f"sq{sh}")
                nc.vector.reduce_sum(out=sq, in_=sqt, axis=mybir.AxisListType.X)
                rsd = per.tile([128, 1], f32, tag=f"rsd{sh}")
                nc.scalar.activation(
                    out=rsd, in_=sq, func=mybir.ActivationFunctionType.Sqrt,
                    bias=epsn[:, 0:1], scale=1.0 / H,
                )
                rt = per.tile([128, 1], f32, tag=f"r{sh}")
                nc.vector.reciprocal(rt, rsd)
                r.append(rt)
                nc.sync.dma_start(
                    out=rb[sh * 128 : (sh + 1) * 128], in_=rt[:, 0:1]
                )

            # broadcast r over partitions: rbcast[p, s] = r[s]
            rbc = per.tile([128, S], f32, tag="rbc")
            rb_ap = bass.AP(tensor=rb.ap().tensor, offset=0, ap=[[0, 128], [1, S]])
            nc.gpsimd.dma_start(out=rbc, in_=rb_ap)

            # ---- P1: normalized h^T in bf16: hTn[p, k, s] = h[s, k*128+p]*r[s]
            hTr = etp.tile([128, KB, S], bf, tag="et")
            nc.sync.dma_start(out=hTr, in_=hT[:].rearrange("k p s -> p k s"))
            hTn = per.tile([128, KB, S], bf, tag="hTn")
            rbc3 = bass.AP(
                tensor=rbc.tensor, offset=rbc.offset,
                ap=[rbc.ap[0], [0, KB], rbc.ap[1]],
            )
            nc.vector.tensor_mul(hTn, hTr, rbc3)

            # ---- P2: gate softmax g (no max shift; logits are O(1)) ----
            gw3 = per.tile([128, KB, E], bf, tag="gw3")
            nc.sync.dma_start(out=gw3, in_=gw[:].rearrange("k p e -> p k e"))
            g = []
            for sh in range(2):
                gps = psS.tile([128, E], f32, tag="ps_small")
                for k in range(KB):
                    nc.tensor.matmul(
                        gps,
                        hTn[:, k, sh * 128 : (sh + 1) * 128],
                        gw3[:, k, :],
                        start=(k == 0),
                        stop=(k == KB - 1),
                    )
                ge = per.tile([128, E], f32, tag=f"ge{sh}")
                gsum = per.tile([128, 1], f32, tag=f"gsum{sh}")
                nc.scalar.activation(
                    out=ge, in_=gps, func=mybir.ActivationFunctionType.Exp,
                    accum_out=gsum[:, 0:1],
                )
                grc = per.tile([128, 1], f32, tag=f"grc{sh}")
                nc.vector.reciprocal(grc, gsum)
                gt = per.tile([128, E], f32, tag=f"g{sh}")
                nc.vector.tensor_scalar_mul(gt, ge, grc[:, 0:1])
                g.append(gt)

            # ---- P3: expert transform shard: 25 (e, dblk) jobs ----
            for j in range(JPC):
                wjt = strm.tile([128, KB, 128], bf, tag="stream")
                nc.sync.dma_start(out=wjt, in_=wj[j].rearrange("k p d -> p k d"))
                bps = psS.tile([128, S], f32, tag="ps_small")
                for k in range(KB):
                    nc.tensor.matmul(
                        bps, wjt[:, k, :], hTn[:, k, :],
                        start=(k == 0), stop=(k == KB - 1),
                    )
                el = ehlp.tile([128, S], bf, tag="ehl")
                nc.vector.tensor_copy(el, bps)
                nc.sync.dma_start(out=ehl[j], in_=el)

            # ---- P4: AllGather expert_hidden^T (bf16, 13 MB) ----
            nc.gpsimd.collective_compute(
                "AllGather", mybir.AluOpType.bypass, replica_groups=rg,
                ins=[ehl[:]], outs=[eha[:]],
            )

            # ---- main: per s-half ----
            for sh in range(2):
                ehsh = ehp.tile([128, NJOBS, 128], bf, tag="ehsh")
                nc.sync.dma_start(
                    out=ehsh,
                    in_=eha[:, :, sh * 128 : (sh + 1) * 128].rearrange(
                        "b p s -> p b s"
                    ),
                )
                zacc = per.tile([128, E], f32, tag=f"zacc{sh}")
                nc.vector.memset(zacc, 0.0)

                # pass 1: logits -> exp -> spill; accumulate Z row-sums
                for v0, vn in CHUNKS:
                    et3 = etp.tile([128, KB, CHUNK], bf, tag="et")
                    nc.sync.dma_start(
                        out=et3[:, :, :vn],
                        in_=embT[:, :, v0 : v0 + vn].rearrange("k p v -> p k v"),
                    )
                    xs3 = xsp_pool.tile([128, E, CHUNK], bf, tag="xs")
                    for e in range(E):
                        cps = psC.tile([128, CHUNK], f32, tag="psC")
                        for k in range(KB):
                            nc.tensor.matmul(
                                cps[:, :vn],
                                ehsh[:, e * KB + k, :],
                                et3[:, k, :vn],
                                start=(k == 0),
                                stop=(k == KB - 1),
                            )
                        zc = per.tile([128, 1], f32, tag="zc", bufs=4)
                        nc.scalar.activation(
                            out=xs3[:, e, :vn], in_=cps[:, :vn],
                            func=mybir.ActivationFunctionType.Exp,
                            accum_out=zc[:, 0:1],
                        )
                        nc.vector.tensor_add(
                            zacc[:, e : e + 1], zacc[:, e : e + 1], zc
                        )
                    nc.sync.dma_start(
                        out=xsp[sh::2, :, v0 : v0 + vn].rearrange("b p v -> p b v"),
                        in_=xs3[:, :, :vn],
                    )

                # Z AllReduce + pad correction + R = g / Z
                nc.sync.dma_start(out=zl[sh][:], in_=zacc)
                nc.gpsimd.collective_compute(
                    "AllReduce", mybir.AluOpType.add, replica_groups=rg,
                    ins=[zl[sh][:]], outs=[za[sh][:]],
                )
                zs = per.tile([128, E], f32, tag=f"zs{sh}")
                nc.sync.dma_start(out=zs, in_=za[sh][:])
                nc.vector.tensor_scalar_add(zs, zs, float(-NPAD))
                zrc = per.tile([128, E], f32, tag=f"zrc{sh}")
                nc.vector.reciprocal(zrc, zs)
                Rt = per.tile([128, E], f32, tag=f"R{sh}")
                nc.vector.tensor_mul(Rt, g[sh], zrc)

                # pass 2: mixed = sum_e R_e * X_e ; out = ln(mixed + eps)
                for v0, vn in CHUNKS:
                    xt3 = strm.tile([128, E, CHUNK], bf, tag="stream")
                    nc.sync.dma_start(
                        out=xt3[:, :, :vn],
                        in_=xsp[sh::2, :, v0 : v0 + vn].rearrange("b p v -> p b v"),
                    )
                    acc = accp.tile([128, CHUNK], f32, tag="acc")
                    nc.vector.tensor_scalar_mul(
                        acc[:, :vn], xt3[:, 0, :vn], Rt[:, 0:1]
                    )
                    for e in range(1, E):
                        nc.vector.scalar_tensor_tensor(
                            out=acc[:, :vn],
                            in0=xt3[:, e, :vn],
                            scalar=Rt[:, e : e + 1],
                            in1=acc[:, :vn],
                            op0=mybir.AluOpType.mult,
                            op1=mybir.AluOpType.add,
                        )
                    ot = otp.tile([128, CHUNK], f32, tag="ot")
                    nc.scalar.activation(
                        out=ot[:, :vn], in_=acc[:, :vn],
                        func=mybir.ActivationFunctionType.Ln,
                        bias=epsl[:, 0:1],
                    )
                    nc.sync.dma_start(
                        out=out[sh * 128 : (sh + 1) * 128, v0 : v0 + vn],
                        in_=ot[:, :vn],
                    )

    nc.compile()
    _nc_cache = nc
    return nc


def prepare_in_maps(inputs):
    h = np.asarray(inputs["hidden_states"], np.float32).reshape(S, H)
    emb = np.asarray(inputs["embedding_matrix"], np.float32)
    ns = np.asarray(inputs["norm_scale"], np.float32)
    W = np.asarray(inputs["expert_weights"], np.float32)
    G = np.asarray(inputs["gate_weight"], np.float32)

    h32 = np.ascontiguousarray(h.reshape(2, 128, H))
    hTb = np.ascontiguousarray(h.T.reshape(KB, 128, S)).astype(BF16)
    gwb = np.ascontiguousarray((G * ns[:, None]).reshape(KB, 128, E)).astype(BF16)

    Wn = W * ns[None, :, None]
    # wjobs_all[j = e*KB + dblk, k] = Wn[e, k*128:(k+1)*128, dblk*128:(dblk+1)*128]
    Wr = Wn.reshape(E, KB, 128, KB, 128)
    wjobs_all = np.ascontiguousarray(
        Wr.transpose(0, 3, 1, 2, 4).reshape(NJOBS, KB, 128, 128)
    ).astype(BF16)

    embp = np.zeros((NCORES * VP, H), np.float32)
    embp[:V] = emb

    in_maps = []
    for c in range(NCORES):
        esl = embp[c * VP : (c + 1) * VP]  # (VP, H)
        embT_c = np.ascontiguousarray(esl.T.reshape(KB, 128, VP)).astype(BF16)
        in_maps.append(
            {
                "h32": h32,
                "hT": hTb,
                "gw": gwb,
                "wjobs": wjobs_all[c * JPC : (c + 1) * JPC],
                "embT": embT_c,
            }
        )
    return in_maps


def assemble_output(results):
    full = np.concatenate([results[c]["out"] for c in range(NCORES)], axis=1)
    return np.ascontiguousarray(full[:, :V].reshape(1, S, V).astype(np.float32))


def kernel(**inputs):
    nc = build_kernel()
    in_maps = prepare_in_maps(inputs)
    res = run_bass_kernel_spmd(nc, in_maps, list(range(NCORES)))
    return assemble_output(res.results)


# revision 2
# speedup vs baseline: 1.0609x; 1.0609x over previous
"""Mixture-of-Softmaxes with shared embedding — 8-core Trainium2 Bass kernel.

Strategy (tensor-parallel on the vocab output head, per the sharding hint):
  - Vocab dim V is sharded across the 8 cores (Vp = 6283 rows each, zero-padded
    from 50257 to 50264; the 7 pad rows contribute exactly exp(0)=1 to each
    softmax denominator and are corrected by a constant subtraction).
  - The expert transforms (10 experts x 2560x2560) are sharded as 200
    (expert, d-block) jobs, 25 per core, followed by an AllGather of the
    bf16 expert_hidden^T (13 MB, split in two collectives to overlap with
    the second half of the expert matmuls).
  - Softmax over the full vocab needs a cross-core reduction: since the
    logits are O(1) in magnitude, exp() cannot overflow in fp32, so the
    max-shift is unnecessary and the reduction collapses to a single
    AllReduce-ADD of the (128,10) sum-of-exp stats per token-half.
  - RMSNorm is folded in linearly: norm_scale is folded into the expert and
    gate weights on the host; the per-token 1/rms factor is computed on-chip
    in fp32 and applied to expert_hidden^T during the PSUM->SBUF copy
    (free-dim broadcast), so the expert matmuls never wait on it.

kernel(**inputs) takes the full unsharded inputs and returns the full
(1, 256, 50257) float32 logits.
"""
import sys

for _p in ("/opt/trn_rl_repo",):
    if _p not in sys.path:
        sys.path.append(_p)

import numpy as np
import ml_dtypes

import concourse.bass as bass
import concourse.bacc as bacc
import concourse.mybir as mybir
import concourse.tile as tile
from concourse.bass_utils import run_bass_kernel_spmd

BF16 = ml_dtypes.bfloat16

NCORES = 8
S = 256          # tokens (B*S)
H = 2560         # hidden
E = 10           # experts
V = 50257        # vocab
KB = H // 128    # 20 k-blocks
NJOBS = E * KB   # 200 (expert, d-block) jobs
JPC = NJOBS // NCORES  # 25 jobs per core
AG1 = 13         # jobs per core in the first AllGather piece
AG2 = JPC - AG1  # 12 in the second
VP = 6283        # per-core vocab slice (8*6283 = 50264)
NPAD = NCORES * VP - V  # 7 zero-pad vocab rows (on the last core)
CHUNK = 512
CHUNKS = [(i * CHUNK, CHUNK) for i in range(VP // CHUNK)]
if VP % CHUNK:
    CHUNKS.append((VP - VP % CHUNK, VP % CHUNK))
EPS_NORM = 1e-05
EPS_LOG = 1e-10

_nc_cache = None


def _eh_runs(e):
    """Contiguous source runs for expert e's 20 k-blocks in the two AG outputs.

    Returns [(buf, row0, k0, n)]: k-blocks k0..k0+n-1 live at rows
    row0..row0+n-1 of AG output `buf` (0 or 1).
    """
    runs = []
    for k in range(KB):
        j = e * KB + k
        c, jj = divmod(j, JPC)
        if jj < AG1:
            buf, row = 0, c * AG1 + jj
        else:
            buf, row = 1, c * AG2 + (jj - AG1)
        if runs and runs[-1][0] == buf and row == runs[-1][1] + runs[-1][3] \
                and k == runs[-1][2] + runs[-1][3]:
            runs[-1][3] += 1
        else:
            runs.append([buf, row, k, 1])
    return runs


def build_kernel():
    global _nc_cache
    if _nc_cache is not None:
        return _nc_cache
    f32 = mybir.dt.float32
    bf = mybir.dt.bfloat16
    nc = bacc.Bacc("TRN2", target_bir_lowering=False, debug=False, num_devices=NCORES)

    h32 = nc.declare_dram_parameter("h32", [2, 128, H], f32, isOutput=False)
    hT = nc.declare_dram_parameter("hT", [KB, 128, S], bf, isOutput=False)
    gw = nc.declare_dram_parameter("gw", [KB, 128, E], bf, isOutput=False)
    wj = nc.declare_dram_parameter("wjobs", [JPC, KB, 128, 128], bf, isOutput=False)
    embT = nc.declare_dram_parameter("embT", [KB, 128, VP], bf, isOutput=False)
    out = nc.declare_dram_parameter("out", [S, VP], f32, isOutput=True)

    rb = nc.dram_tensor("rbounce", [S], f32)
    ehl1 = nc.dram_tensor("eh_local1", [AG1, 128, S], bf)
    ehl2 = nc.dram_tensor("eh_local2", [AG2, 128, S], bf)
    eha1 = nc.dram_tensor("eh_all1", [NCORES * AG1, 128, S], bf, addr_space="Shared")
    eha2 = nc.dram_tensor("eh_all2", [NCORES * AG2, 128, S], bf, addr_space="Shared")
    eha = [eha1, eha2]
    zl = [nc.dram_tensor(f"zl{sh}", [128, E], f32) for sh in range(2)]
    za = [
        nc.dram_tensor(f"za{sh}", [128, E], f32, addr_space="Shared")
        for sh in range(2)
    ]
    xsp = nc.dram_tensor("xspill", [2 * E, 128, VP], bf)

    rg = [list(range(NCORES))]

    with tile.TileContext(nc) as tc:
        with (
            tc.tile_pool(name="et", bufs=2) as etp,          # 20KB slots
            tc.tile_pool(name="stream", bufs=3) as strm,     # 10KB slots
            tc.tile_pool(name="xs", bufs=2) as xsp_pool,     # 10KB slots
            tc.tile_pool(name="ehsh", bufs=1) as ehp,        # 10 x 5.12KB
            tc.tile_pool(name="acc", bufs=2) as accp,
            tc.tile_pool(name="ot", bufs=2) as otp,
            tc.tile_pool(name="ehl", bufs=4) as ehlp,
            tc.tile_pool(name="persist", bufs=1) as per,
            tc.tile_pool(name="psmall", bufs=4, space="PSUM") as psS,
            tc.tile_pool(name="psbig", bufs=4, space="PSUM") as psC,
        ):
            epsn = per.tile([128, 1], f32, tag="epsn")
            nc.vector.memset(epsn, EPS_NORM)
            epsl = per.tile([128, 1], f32, tag="epsl")
            nc.vector.memset(epsl, EPS_LOG)

            # raw h^T (bf16) — expert matmuls use it un-normalized
            hTr = per.tile([128, KB, S], bf, tag="hTr")
            nc.sync.dma_start(out=hTr, in_=hT[:].rearrange("k p s -> p k s"))

            # ---- per-token RMS factors r_s = 1/sqrt(mean(h^2)+eps) ----
            r = []
            for sh in range(2):
                ht = etp.tile([128, H], f32, tag="et")
                nc.sync.dma_start(out=ht, in_=h32[sh])
                sqf = etp.tile([128, H], f32, tag="et")
                sq = per.tile([128, 1], f32, tag=f"sq{sh}")
                nc.scalar.activation(
                    out=sqf, in_=ht, func=mybir.ActivationFunctionType.Square,
                    accum_out=sq[:, 0:1],
                )
                rsd = per.tile([128, 1], f32, tag=f"rsd{sh}")
                nc.scalar.activation(
                    out=rsd, in_=sq, func=mybir.ActivationFunctionType.Sqrt,
                    bias=epsn[:, 0:1], scale=1.0 / H,
                )
                rt = per.tile([128, 1], f32, tag=f"r{sh}")
                nc.vector.reciprocal(rt, rsd)
                r.append(rt)
                nc.sync.dma_start(
                    out=rb[sh * 128 : (sh + 1) * 128], in_=rt[:, 0:1]
                )

            # broadcast r over partitions: rbc[p, s] = r[s]
            rbc = per.tile([128, S], f32, tag="rbc")
            rb_ap = bass.AP(tensor=rb.ap().tensor, offset=0, ap=[[0, 128], [1, S]])
            nc.gpsimd.dma_start(out=rbc, in_=rb_ap)

            # ---- expert transform shard: 25 (e, dblk) jobs; r applied in copy
            for j in range(JPC):
                wjt = strm.tile([128, KB, 128], bf, tag="stream")
                nc.sync.dma_start(out=wjt, in_=wj[j].rearrange("k p d -> p k d"))
                bps = psS.tile([128, S], f32, tag="ps_small")
                for k in range(KB):
                    nc.tensor.matmul(
                        bps, wjt[:, k, :], hTr[:, k, :],
                        start=(k == 0), stop=(k == KB - 1),
                    )
                el = ehlp.tile([128, S], bf, tag="ehl")
                nc.vector.tensor_mul(el, bps, rbc)
                if j < AG1:
                    nc.sync.dma_start(out=ehl1[j], in_=el)
                else:
                    nc.sync.dma_start(out=ehl2[j - AG1], in_=el)
                if j == AG1 - 1:
                    nc.gpsimd.collective_compute(
                        "AllGather", mybir.AluOpType.bypass, replica_groups=rg,
                        ins=[ehl1[:]], outs=[eha1[:]],
                    )
            nc.gpsimd.collective_compute(
                "AllGather", mybir.AluOpType.bypass, replica_groups=rg,
                ins=[ehl2[:]], outs=[eha2[:]],
            )

            # ---- gate softmax g (no max shift; logits are O(1)) ----
            gw3 = per.tile([128, KB, E], bf, tag="gw3")
            nc.sync.dma_start(out=gw3, in_=gw[:].rearrange("k p e -> p k e"))
            g = []
            for sh in range(2):
                gps = psS.tile([128, E], f32, tag="ps_small")
                for k in range(KB):
                    nc.tensor.matmul(
                        gps,
                        hTr[:, k, sh * 128 : (sh + 1) * 128],
                        gw3[:, k, :],
                        start=(k == 0),
                        stop=(k == KB - 1),
                    )
                ge = per.tile([128, E], f32, tag=f"ge{sh}")
                gsum = per.tile([128, 1], f32, tag=f"gsum{sh}")
                nc.scalar.activation(
                    out=ge, in_=gps, func=mybir.ActivationFunctionType.Exp,
                    scale=r[sh][:, 0:1], accum_out=gsum[:, 0:1],
                )
                grc = per.tile([128, 1], f32, tag=f"grc{sh}")
                nc.vector.reciprocal(grc, gsum)
                gt = per.tile([128, E], f32, tag=f"g{sh}")
                nc.vector.tensor_scalar_mul(gt, ge, grc[:, 0:1])
                g.append(gt)

            # ---- main: per s-half ----
            for sh in range(2):
                ehsh = []
                for e in range(E):
                    te = ehp.tile([128, KB, 128], bf, tag=f"ehsh{e}")
                    for buf, row0, k0, n in _eh_runs(e):
                        nc.sync.dma_start(
                            out=te[:, k0 : k0 + n, :],
                            in_=eha[buf][
                                row0 : row0 + n, :, sh * 128 : (sh + 1) * 128
                            ].rearrange("b p s -> p b s"),
                        )
                    ehsh.append(te)
                zacc = per.tile([128, E], f32, tag=f"zacc{sh}")
                nc.vector.memset(zacc, 0.0)

                # pass 1: logits -> exp -> spill; accumulate Z row-sums
                for v0, vn in CHUNKS:
                    et3 = etp.tile([128, KB, CHUNK], bf, tag="et")
                    nc.sync.dma_start(
                        out=et3[:, :, :vn],
                        in_=embT[:, :, v0 : v0 + vn].rearrange("k p v -> p k v"),
                    )
                    xs3 = xsp_pool.tile([128, E, CHUNK], bf, tag="xs")
                    for e in range(E):
                        cps = psC.tile([128, CHUNK], f32, tag="psC")
                        for k in range(KB):
                            nc.tensor.matmul(
                                cps[:, :vn],
                                ehsh[e][:, k, :],
                                et3[:, k, :vn],
                                start=(k == 0),
                                stop=(k == KB - 1),
                            )
                        zc = per.tile([128, 1], f32, tag="zc", bufs=4)
                        nc.scalar.activation(
                            out=xs3[:, e, :vn], in_=cps[:, :vn],
                            func=mybir.ActivationFunctionType.Exp,
                            accum_out=zc[:, 0:1],
                        )
                        nc.vector.tensor_add(
                            zacc[:, e : e + 1], zacc[:, e : e + 1], zc
                        )
                    nc.sync.dma_start(
                        out=xsp[sh::2, :, v0 : v0 + vn].rearrange("b p v -> p b v"),
                        in_=xs3[:, :, :vn],
                    )

                # Z AllReduce + pad correction + R = g / Z
                nc.sync.dma_start(out=zl[sh][:], in_=zacc)
                nc.gpsimd.collective_compute(
                    "AllReduce", mybir.AluOpType.add, replica_groups=rg,
                    ins=[zl[sh][:]], outs=[za[sh][:]],
                )
                zs = per.tile([128, E], f32, tag=f"zs{sh}")
                nc.sync.dma_start(out=zs, in_=za[sh][:])
                nc.vector.tensor_scalar_add(zs, zs, float(-NPAD))
                zrc = per.tile([128, E], f32, tag=f"zrc{sh}")
                nc.vector.reciprocal(zrc, zs)
                Rt = per.tile([128, E], f32, tag=f"R{sh}")
                nc.vector.tensor_mul(Rt, g[sh], zrc)

                # pass 2: mixed = sum_e R_e * X_e ; out = ln(mixed + eps)
                for v0, vn in CHUNKS:
                    xt3 = strm.tile([128, E, CHUNK], bf, tag="stream")
                    nc.sync.dma_start(
                        out=xt3[:, :, :vn],
                        in_=xsp[sh::2, :, v0 : v0 + vn].rearrange("b p v -> p b v"),
                    )
                    acc = accp.tile([128, CHUNK], f32, tag="acc")
                    nc.vector.tensor_scalar_mul(
                        acc[:, :vn], xt3[:, 0, :vn], Rt[:, 0:1]
                    )
                    for e in range(1, E):
                        nc.vector.scalar_tensor_tensor(
                            out=acc[:, :vn],
                            in0=xt3[:, e, :vn],
                            scalar=Rt[:, e : e + 1],
                            in1=acc[:, :vn],
                            op0=mybir.AluOpType.mult,
                            op1=mybir.AluOpType.add,
                        )
                    ot = otp.tile([128, CHUNK], f32, tag="ot")
                    nc.scalar.activation(
                        out=ot[:, :vn], in_=acc[:, :vn],
                        func=mybir.ActivationFunctionType.Ln,
                        bias=epsl[:, 0:1],
                    )
                    nc.sync.dma_start(
                        out=out[sh * 128 : (sh + 1) * 128, v0 : v0 + vn],
                        in_=ot[:, :vn],
                    )

    nc.compile()
    _nc_cache = nc
    return nc


def prepare_in_maps(inputs):
    h = np.asarray(inputs["hidden_states"], np.float32).reshape(S, H)
    emb = np.asarray(inputs["embedding_matrix"], np.float32)
    ns = np.asarray(inputs["norm_scale"], np.float32)
    W = np.asarray(inputs["expert_weights"], np.float32)
    G = np.asarray(inputs["gate_weight"], np.float32)

    h32 = np.ascontiguousarray(h.reshape(2, 128, H))
    hTb = np.ascontiguousarray(h.T.reshape(KB, 128, S)).astype(BF16)
    gwb = np.ascontiguousarray((G * ns[:, None]).reshape(KB, 128, E)).astype(BF16)

    Wn = W * ns[None, :, None]
    # wjobs_all[j = e*KB + dblk, k] = Wn[e, k*128:(k+1)*128, dblk*128:(dblk+1)*128]
    Wr = Wn.reshape(E, KB, 128, KB, 128)
    wjobs_all = np.ascontiguousarray(
        Wr.transpose(0, 3, 1, 2, 4).reshape(NJOBS, KB, 128, 128)
    ).astype(BF16)

    embp = np.zeros((NCORES * VP, H), np.float32)
    embp[:V] = emb

    in_maps = []
    for c in range(NCORES):
        esl = embp[c * VP : (c + 1) * VP]  # (VP, H)
        embT_c = np.ascontiguousarray(esl.T.reshape(KB, 128, VP)).astype(BF16)
        in_maps.append(
            {
                "h32": h32,
                "hT": hTb,
                "gw": gwb,
                "wjobs": wjobs_all[c * JPC : (c + 1) * JPC],
                "embT": embT_c,
            }
        )
    return in_maps


def assemble_output(results):
    full = np.concatenate([results[c]["out"] for c in range(NCORES)], axis=1)
    return np.ascontiguousarray(full[:, :V].reshape(1, S, V).astype(np.float32))


def kernel(**inputs):
    nc = build_kernel()
    in_maps = prepare_in_maps(inputs)
    res = run_bass_kernel_spmd(nc, in_maps, list(range(NCORES)))
    return assemble_output(res.results)


# revision 3
# speedup vs baseline: 1.7215x; 1.6226x over previous
"""Mixture-of-Softmaxes with shared embedding — 8-core Trainium2 Bass kernel.

Strategy (tensor-parallel on the vocab output head, per the sharding hint):
  - Vocab dim V is sharded across the 8 cores (Vp = 6283 rows each, zero-padded
    from 50257 to 50264; the 7 pad rows contribute exactly exp(0)=1 to each
    softmax denominator and are corrected by a constant subtraction).
  - The expert transforms (10 experts x 2560x2560) are sharded as 200
    (expert, d-block) jobs, 25 per core, followed by an AllGather of the
    bf16 expert_hidden^T (13 MB, split in two collectives to overlap with
    the second half of the expert matmuls).
  - Softmax over the full vocab needs a cross-core reduction: since the
    logits are O(1) in magnitude, exp() cannot overflow in fp32, so the
    max-shift is unnecessary and the reduction collapses to a single
    AllReduce-ADD of the (128,10) sum-of-exp stats per token-half.
  - RMSNorm is folded in linearly: norm_scale is folded into the expert and
    gate weights on the host; the per-token 1/rms factor is computed on-chip
    in fp32 and applied to expert_hidden^T during the PSUM->SBUF copy
    (free-dim broadcast), so the expert matmuls never wait on it.

kernel(**inputs) takes the full unsharded inputs and returns the full
(1, 256, 50257) float32 logits.
"""
import sys

for _p in ("/opt/trn_rl_repo",):
    if _p not in sys.path:
        sys.path.append(_p)

import numpy as np
import ml_dtypes

import concourse.bass as bass
import concourse.bacc as bacc
import concourse.mybir as mybir
import concourse.tile as tile
from concourse.bass_utils import run_bass_kernel_spmd

BF16 = ml_dtypes.bfloat16

NCORES = 8
S = 256          # tokens (B*S)
H = 2560         # hidden
E = 10           # experts
V = 50257        # vocab
KB = H // 128    # 20 k-blocks
NJOBS = E * KB   # 200 (expert, d-block) jobs
JPC = NJOBS // NCORES  # 25 jobs per core
AG1 = 13         # jobs per core in the first AllGather piece
AG2 = JPC - AG1  # 12 in the second
VP = 6283        # per-core vocab slice (8*6283 = 50264)
NPAD = NCORES * VP - V  # 7 zero-pad vocab rows (on the last core)
CHUNK = 512
CHUNKS = [(i * CHUNK, CHUNK) for i in range(VP // CHUNK)]
if VP % CHUNK:
    CHUNKS.append((VP - VP % CHUNK, VP % CHUNK))
EPS_NORM = 1e-05
EPS_LOG = 1e-10
FP8 = ml_dtypes.float8_e4m3
EMB_SCALE = 64.0
EH_SCALE = 16.0
INV_SCALE = 1.0 / (EMB_SCALE * EH_SCALE)

_nc_cache = None


def _eh_runs(e):
    """Contiguous source runs for expert e's 20 k-blocks in the two AG outputs.

    Returns [(buf, row0, k0, n)]: k-blocks k0..k0+n-1 live at rows
    row0..row0+n-1 of AG output `buf` (0 or 1).
    """
    runs = []
    for k in range(KB):
        j = e * KB + k
        c, jj = divmod(j, JPC)
        if jj < AG1:
            buf, row = 0, c * AG1 + jj
        else:
            buf, row = 1, c * AG2 + (jj - AG1)
        if runs and runs[-1][0] == buf and row == runs[-1][1] + runs[-1][3] \
                and k == runs[-1][2] + runs[-1][3]:
            runs[-1][3] += 1
        else:
            runs.append([buf, row, k, 1])
    return runs


def build_kernel():
    global _nc_cache
    if _nc_cache is not None:
        return _nc_cache
    f32 = mybir.dt.float32
    bf = mybir.dt.bfloat16
    f8 = mybir.dt.float8e4
    nc = bacc.Bacc("TRN2", target_bir_lowering=False, debug=False, num_devices=NCORES)

    h32 = nc.declare_dram_parameter("h32", [2, 128, H], f32, isOutput=False)
    hT = nc.declare_dram_parameter("hT", [KB, 128, S], bf, isOutput=False)
    gw = nc.declare_dram_parameter("gw", [KB, 128, E], bf, isOutput=False)
    wj = nc.declare_dram_parameter("wjobs", [JPC, KB, 128, 128], bf, isOutput=False)
    embT = nc.declare_dram_parameter("embT", [KB, 128, VP], f8, isOutput=False)
    out = nc.declare_dram_parameter("out", [S, VP], f32, isOutput=True)

    rb = nc.dram_tensor("rbounce", [S], f32)
    ehl1 = nc.dram_tensor("eh_local1", [AG1, 128, S], f8)
    ehl2 = nc.dram_tensor("eh_local2", [AG2, 128, S], f8)
    eha1 = nc.dram_tensor("eh_all1", [NCORES * AG1, 128, S], f8, addr_space="Shared")
    eha2 = nc.dram_tensor("eh_all2", [NCORES * AG2, 128, S], f8, addr_space="Shared")
    eha = [eha1, eha2]
    zl = [nc.dram_tensor(f"zl{sh}", [128, E], f32) for sh in range(2)]
    za = [
        nc.dram_tensor(f"za{sh}", [128, E], f32, addr_space="Shared")
        for sh in range(2)
    ]
    xsp = nc.dram_tensor("xspill", [2 * E, 128, VP], bf)

    rg = [list(range(NCORES))]

    with tile.TileContext(nc) as tc:
        with (
            tc.tile_pool(name="et", bufs=2) as etp,          # 20KB slots
            tc.tile_pool(name="stream", bufs=3) as strm,     # 10KB slots
            tc.tile_pool(name="xs", bufs=2) as xsp_pool,     # 10KB slots
            tc.tile_pool(name="ehsh", bufs=1) as ehp,        # 10 x 5.12KB
            tc.tile_pool(name="acc", bufs=2) as accp,
            tc.tile_pool(name="ot", bufs=2) as otp,
            tc.tile_pool(name="ehl", bufs=4) as ehlp,
            tc.tile_pool(name="persist", bufs=1) as per,
            tc.tile_pool(name="psmall", bufs=4, space="PSUM") as psS,
            tc.tile_pool(name="psbig", bufs=4, space="PSUM") as psC,
        ):
            epsn = per.tile([128, 1], f32, tag="epsn")
            nc.vector.memset(epsn, EPS_NORM)
            epsl = per.tile([128, 1], f32, tag="epsl")
            nc.vector.memset(epsl, EPS_LOG)

            # raw h^T (bf16) — expert matmuls use it un-normalized
            hTr = per.tile([128, KB, S], bf, tag="hTr")
            nc.sync.dma_start(out=hTr, in_=hT[:].rearrange("k p s -> p k s"))

            # ---- per-token RMS factors r_s = 1/sqrt(mean(h^2)+eps) ----
            r = []
            for sh in range(2):
                ht = etp.tile([128, H], f32, tag="et")
                nc.sync.dma_start(out=ht, in_=h32[sh])
                sqf = etp.tile([128, H], f32, tag="et")
                sq = per.tile([128, 1], f32, tag=f"sq{sh}")
                nc.scalar.activation(
                    out=sqf, in_=ht, func=mybir.ActivationFunctionType.Square,
                    accum_out=sq[:, 0:1],
                )
                rsd = per.tile([128, 1], f32, tag=f"rsd{sh}")
                nc.scalar.activation(
                    out=rsd, in_=sq, func=mybir.ActivationFunctionType.Sqrt,
                    bias=epsn[:, 0:1], scale=1.0 / H,
                )
                rt = per.tile([128, 1], f32, tag=f"r{sh}")
                nc.vector.reciprocal(rt, rsd)
                r.append(rt)
                nc.sync.dma_start(
                    out=rb[sh * 128 : (sh + 1) * 128], in_=rt[:, 0:1]
                )

            # broadcast r over partitions: rbc[p, s] = r[s]
            rbc = per.tile([128, S], f32, tag="rbc")
            rb_ap = bass.AP(tensor=rb.ap().tensor, offset=0, ap=[[0, 128], [1, S]])
            nc.gpsimd.dma_start(out=rbc, in_=rb_ap)
            rbc16 = per.tile([128, S], f32, tag="rbc16")
            nc.vector.tensor_scalar_mul(rbc16, rbc, EH_SCALE)

            # ---- expert transform shard: 25 (e, dblk) jobs; r applied in copy
            for j in range(JPC):
                wjt = strm.tile([128, KB, 128], bf, tag="stream")
                nc.sync.dma_start(out=wjt, in_=wj[j].rearrange("k p d -> p k d"))
                bps = psS.tile([128, S], f32, tag="ps_small")
                for k in range(KB):
                    nc.tensor.matmul(
                        bps, wjt[:, k, :], hTr[:, k, :],
                        start=(k == 0), stop=(k == KB - 1),
                    )
                el = ehlp.tile([128, S], f8, tag="ehl")
                nc.vector.tensor_mul(el, bps, rbc16)
                if j < AG1:
                    nc.sync.dma_start(out=ehl1[j], in_=el)
                else:
                    nc.sync.dma_start(out=ehl2[j - AG1], in_=el)
                if j == AG1 - 1:
                    nc.gpsimd.collective_compute(
                        "AllGather", mybir.AluOpType.bypass, replica_groups=rg,
                        ins=[ehl1[:]], outs=[eha1[:]],
                    )
            nc.gpsimd.collective_compute(
                "AllGather", mybir.AluOpType.bypass, replica_groups=rg,
                ins=[ehl2[:]], outs=[eha2[:]],
            )

            # ---- gate softmax g (no max shift; logits are O(1)) ----
            gw3 = per.tile([128, KB, E], bf, tag="gw3")
            nc.sync.dma_start(out=gw3, in_=gw[:].rearrange("k p e -> p k e"))
            g = []
            for sh in range(2):
                gps = psS.tile([128, E], f32, tag="ps_small")
                for k in range(KB):
                    nc.tensor.matmul(
                        gps,
                        hTr[:, k, sh * 128 : (sh + 1) * 128],
                        gw3[:, k, :],
                        start=(k == 0),
                        stop=(k == KB - 1),
                    )
                ge = per.tile([128, E], f32, tag=f"ge{sh}")
                gsum = per.tile([128, 1], f32, tag=f"gsum{sh}")
                nc.scalar.activation(
                    out=ge, in_=gps, func=mybir.ActivationFunctionType.Exp,
                    scale=r[sh][:, 0:1], accum_out=gsum[:, 0:1],
                )
                grc = per.tile([128, 1], f32, tag=f"grc{sh}")
                nc.vector.reciprocal(grc, gsum)
                gt = per.tile([128, E], f32, tag=f"g{sh}")
                nc.vector.tensor_scalar_mul(gt, ge, grc[:, 0:1])
                g.append(gt)

            # ---- main: per s-half ----
            for sh in range(2):
                ehsh = []
                for e in range(E):
                    te = ehp.tile([128, KB, 128], f8, tag=f"ehsh{e}")
                    for buf, row0, k0, n in _eh_runs(e):
                        nc.sync.dma_start(
                            out=te[:, k0 : k0 + n, :],
                            in_=eha[buf][
                                row0 : row0 + n, :, sh * 128 : (sh + 1) * 128
                            ].rearrange("b p s -> p b s"),
                        )
                    ehsh.append(te)
                zacc = per.tile([128, E], f32, tag=f"zacc{sh}")
                nc.vector.memset(zacc, 0.0)

                # pass 1: logits -> exp -> spill; accumulate Z row-sums
                for v0, vn in CHUNKS:
                    et3 = etp.tile([128, KB, CHUNK], f8, tag="et")
                    nc.sync.dma_start(
                        out=et3[:, :, :vn],
                        in_=embT[:, :, v0 : v0 + vn].rearrange("k p v -> p k v"),
                    )
                    xs3 = xsp_pool.tile([128, E, CHUNK], bf, tag="xs")
                    for e in range(E):
                        cps = psC.tile([128, CHUNK], f32, tag="psC")
                        for k2 in range(KB // 2):
                            nc.tensor.matmul(
                                cps[:, :vn],
                                ehsh[e][:, 2 * k2 : 2 * k2 + 2, :],
                                et3[:, 2 * k2 : 2 * k2 + 2, :vn],
                                start=(k2 == 0),
                                stop=(k2 == KB // 2 - 1),
                                perf_mode=mybir.MatmulPerfMode.DoubleRow,
                            )
                        zc = per.tile([128, 1], f32, tag="zc", bufs=4)
                        nc.scalar.activation(
                            out=xs3[:, e, :vn], in_=cps[:, :vn],
                            func=mybir.ActivationFunctionType.Exp,
                            scale=INV_SCALE,
                            accum_out=zc[:, 0:1],
                        )
                        nc.vector.tensor_add(
                            zacc[:, e : e + 1], zacc[:, e : e + 1], zc
                        )
                    nc.sync.dma_start(
                        out=xsp[sh::2, :, v0 : v0 + vn].rearrange("b p v -> p b v"),
                        in_=xs3[:, :, :vn],
                    )

                # Z AllReduce + pad correction + R = g / Z
                nc.sync.dma_start(out=zl[sh][:], in_=zacc)
                nc.gpsimd.collective_compute(
                    "AllReduce", mybir.AluOpType.add, replica_groups=rg,
                    ins=[zl[sh][:]], outs=[za[sh][:]],
                )
                zs = per.tile([128, E], f32, tag=f"zs{sh}")
                nc.sync.dma_start(out=zs, in_=za[sh][:])
                nc.vector.tensor_scalar_add(zs, zs, float(-NPAD))
                zrc = per.tile([128, E], f32, tag=f"zrc{sh}")
                nc.vector.reciprocal(zrc, zs)
                Rt = per.tile([128, E], f32, tag=f"R{sh}")
                nc.vector.tensor_mul(Rt, g[sh], zrc)

                # pass 2: mixed = sum_e R_e * X_e ; out = ln(mixed + eps)
                for v0, vn in CHUNKS:
                    xt3 = strm.tile([128, E, CHUNK], bf, tag="stream")
                    nc.sync.dma_start(
                        out=xt3[:, :, :vn],
                        in_=xsp[sh::2, :, v0 : v0 + vn].rearrange("b p v -> p b v"),
                    )
                    acc = accp.tile([128, CHUNK], f32, tag="acc")
                    nc.vector.tensor_scalar_mul(
                        acc[:, :vn], xt3[:, 0, :vn], Rt[:, 0:1]
                    )
                    for e in range(1, E):
                        nc.vector.scalar_tensor_tensor(
                            out=acc[:, :vn],
                            in0=xt3[:, e, :vn],
                            scalar=Rt[:, e : e + 1],
                            in1=acc[:, :vn],
                            op0=mybir.AluOpType.mult,
                            op1=mybir.AluOpType.add,
                        )
                    ot = otp.tile([128, CHUNK], f32, tag="ot")
                    nc.scalar.activation(
                        out=ot[:, :vn], in_=acc[:, :vn],
                        func=mybir.ActivationFunctionType.Ln,
                        bias=epsl[:, 0:1],
                    )
                    nc.sync.dma_start(
                        out=out[sh * 128 : (sh + 1) * 128, v0 : v0 + vn],
                        in_=ot[:, :vn],
                    )

    nc.compile()
    _nc_cache = nc
    return nc


def prepare_in_maps(inputs):
    h = np.asarray(inputs["hidden_states"], np.float32).reshape(S, H)
    emb = np.asarray(inputs["embedding_matrix"], np.float32)
    ns = np.asarray(inputs["norm_scale"], np.float32)
    W = np.asarray(inputs["expert_weights"], np.float32)
    G = np.asarray(inputs["gate_weight"], np.float32)

    h32 = np.ascontiguousarray(h.reshape(2, 128, H))
    hTb = np.ascontiguousarray(h.T.reshape(KB, 128, S)).astype(BF16)
    gwb = np.ascontiguousarray((G * ns[:, None]).reshape(KB, 128, E)).astype(BF16)

    Wn = W * ns[None, :, None]
    # wjobs_all[j = e*KB + dblk, k] = Wn[e, k*128:(k+1)*128, dblk*128:(dblk+1)*128]
    Wr = Wn.reshape(E, KB, 128, KB, 128)
    wjobs_all = np.ascontiguousarray(
        Wr.transpose(0, 3, 1, 2, 4).reshape(NJOBS, KB, 128, 128)
    ).astype(BF16)

    embp = np.zeros((NCORES * VP, H), np.float32)
    embp[:V] = emb

    in_maps = []
    for c in range(NCORES):
        esl = embp[c * VP : (c + 1) * VP]  # (VP, H)
        embT_c = (np.ascontiguousarray(esl.T.reshape(KB, 128, VP)) * EMB_SCALE).astype(FP8)
        in_maps.append(
            {
                "h32": h32,
                "hT": hTb,
                "gw": gwb,
                "wjobs": wjobs_all[c * JPC : (c + 1) * JPC],
                "embT": embT_c,
            }
        )
    return in_maps


def assemble_output(results):
    full = np.concatenate([results[c]["out"] for c in range(NCORES)], axis=1)
    return np.ascontiguousarray(full[:, :V].reshape(1, S, V).astype(np.float32))


def kernel(**inputs):
    nc = build_kernel()
    in_maps = prepare_in_maps(inputs)
    res = run_bass_kernel_spmd(nc, in_maps, list(range(NCORES)))
    return assemble_output(res.results)
